# revision 45
# baseline (speedup 1.0000x reference)
"""Trainium2 Bass kernel for nn_GAT_65231963291731.

GAT layer (4 heads x 16) + graph max-pool + linear classifier over a
100K-node / 3.3M-edge batch of 512 graphs, dst-sharded across 8 NeuronCores.

Factorization: out[d] = (sum_e p_e * x4[src_e]) @ W4 with x4 = (x, 1), so the
device aggregates 4 values per (edge, head) instead of 64, and all per-edge
math is dense matmuls with constant weight matrices:
  e4   = blockdiag(As|Ad) applied to host-gathered x4[src]/x4[dst] tiles (PE)
  p4   = exp(leaky_relu(e4))             (ScalarE Prelu alpha=0.2, then Exp)
  q16  = p4 (x) x4src                    (VectorE, stride-0 broadcast APs)
  Q4   = Sel^T @ q16                     (PE, constant run-selector lhsT)
then a node-scale phase: normalize by s (=Q4[:,:,3]), apply W, bias+relu,
per-graph max-pool, classifier. Host work is index/gather/layout only.
"""
import numpy as np
import ml_dtypes

N, IN_DIM, H, C, HC, OUT_DIM, G = 100000, 3, 4, 16, 64, 2, 512
NC = 8
DSH = N // NC            # 12500 owned dsts per core
D = 12544                # padded dst grid (= 49 * 256)
L = 64                   # edge slots per dst
T = D // 256             # 49 supertiles of 256 dsts = 16384 edges
NEG = 0.2
BF = ml_dtypes.bfloat16

_PROG = None


def _build_program(gstarts):
    from concourse import bacc, mybir, tile

    def act_recip(nc, out_ap, in_ap):
        eng = nc.scalar
        ins = [eng.lower_ap(in_ap)]
        for arg in (0.0, 1.0, 0.0):  # bias, scale, alpha
            ins.append(mybir.ImmediateValue(dtype=mybir.dt.float32, value=arg))
        return eng.add_instruction(mybir.InstActivation(
            name=nc.get_next_instruction_name(),
            func=mybir.ActivationFunctionType.Reciprocal,
            ins=ins, outs=[eng.lower_ap(out_ap)]))

    nc = bacc.Bacc("TRN2", target_bir_lowering=False, debug=False)
    dt = mybir.dt
    AF = mybir.ActivationFunctionType

    xsdt_d = nc.dram_tensor("xsdt", [T, 128, 1024], dt.bfloat16, kind="ExternalInput")
    x4st_d = nc.dram_tensor("x4st", [T, 128, 512], dt.bfloat16, kind="ExternalInput")
    rhse4_d = nc.dram_tensor("rhse4", [128, 64], dt.bfloat16, kind="ExternalInput")
    selt_d = nc.dram_tensor("selt", [128, 32], dt.bfloat16, kind="ExternalInput")
    wrs_d = nc.dram_tensor("wrs", [16, 128], dt.bfloat16, kind="ExternalInput")
    bias_d = nc.dram_tensor("biasv", [64, 1], dt.float32, kind="ExternalInput")
    clfw_d = nc.dram_tensor("clfw", [64, 2], dt.float32, kind="ExternalInput")
    clfb_d = nc.dram_tensor("clfb", [64, 2], dt.float32, kind="ExternalInput")
    out_d = nc.dram_tensor("out", [64, 2], dt.float32, kind="ExternalOutput")
    TQ = [(0, 12), (12, 12), (24, 12), (36, 13)]
    q4hbms = [nc.dram_tensor(f"q4hbm{k}", [8, tn * 512], dt.float32)
              for k, (t0, tn) in enumerate(TQ)]

    with tile.TileContext(nc) as tc:
        with tc.tile_pool(name="const", bufs=1) as cpool, \
             tc.tile_pool(name="stream", bufs=2) as spool, \
             tc.tile_pool(name="work", bufs=4) as wpool, \
             tc.tile_pool(name="acc", bufs=1) as apool, \
             tc.tile_pool(name="psA", bufs=2, space="PSUM") as psA, \
             tc.tile_pool(name="psB", bufs=2, space="PSUM") as psB, \
             tc.tile_pool(name="psN", bufs=2, space="PSUM") as psN:

            rhse4 = cpool.tile([128, 64], dt.bfloat16, tag="rhse4")
            selt = cpool.tile([128, 32], dt.bfloat16, tag="selt")
            wrs = cpool.tile([16, 128], dt.bfloat16, tag="wrs")
            biasv = cpool.tile([64, 1], dt.float32, tag="biasv")
            clfw = cpool.tile([64, 2], dt.float32, tag="clfw")
            clfb = cpool.tile([64, 2], dt.float32, tag="clfb")
            for t_, d_ in [(rhse4, rhse4_d), (selt, selt_d), (wrs, wrs_d),
                           (biasv, bias_d), (clfw, clfw_d), (clfb, clfb_d)]:
                nc.sync.dma_start(out=t_[:], in_=d_[:])

            # ---- edge phase (stream 4 supertiles per DMA) ----
            groups = [(g * 4, min(4, T - g * 4)) for g in range((T + 3) // 4)]
            for (t0, tn) in groups:
                xsdt = spool.tile([128, 4096], dt.bfloat16, tag="xsdt")
                x4st = spool.tile([128, 2048], dt.bfloat16, tag="x4st")
                nc.sync.dma_start(
                    out=xsdt[:, :tn * 1024],
                    in_=xsdt_d[t0:t0 + tn].transpose([1, 0, 2]))
                nc.sync.dma_start(
                    out=x4st[:, :tn * 512],
                    in_=x4st_d[t0:t0 + tn].transpose([1, 0, 2]))
                q4g = wpool.tile([8, 2048], dt.float32, tag="q4g")
                for ti in range(tn):
                    t = t0 + ti
                    e4 = psA.tile([128, 512], dt.float32, tag="e4", space="PSUM")
                    for b in range(8):
                        nc.tensor.matmul(
                            out=e4[:, b * 64:(b + 1) * 64],
                            lhsT=xsdt[:, ti * 1024 + b * 128:ti * 1024 + (b + 1) * 128],
                            rhs=rhse4[:], start=True, stop=True)
                    l4 = wpool.tile([128, 512], dt.bfloat16, tag="l4")
                    # Prelu writes l4 h-outer bf16: col = h*128 + b*16 + g
                    l4o = l4[:].rearrange("p (h b g) -> p b g h", h=4, b=8, g=16)
                    e4v = e4[:].rearrange("p (b g h) -> p b g h", b=8, g=16, h=4)
                    nc.scalar.activation(l4o, e4v, AF.Prelu, 0.0, 1.0, NEG)
                    p4 = wpool.tile([128, 512], dt.bfloat16, tag="p4")
                    nc.scalar.activation(p4[:], l4[:], AF.Exp)

                    # q16 cols (h, i, bg); every operand's last dim is stride-1
                    q16 = wpool.tile([128, 2048], dt.bfloat16, tag="q16")
                    p4b = p4[:].rearrange("p (h bg) -> p h bg", h=4).unsqueeze(2)                         .to_broadcast([128, 4, 4, 128])
                    x4b = x4st[:, ti * 512:(ti + 1) * 512]                         .rearrange("p (i bg) -> p i bg", i=4).unsqueeze(1)                         .to_broadcast([128, 4, 4, 128])
                    q16v = q16[:].rearrange("p (h i bg) -> p h i bg", h=4, i=4)
                    nc.vector.tensor_tensor(out=q16v, in0=p4b, in1=x4b,
                                            op=mybir.AluOpType.mult)

                    q4p = psB.tile([8, 512], dt.float32, tag="q4p", space="PSUM")
                    for sg in range(4):
                        nc.tensor.matmul(out=q4p[:],
                                         lhsT=selt[:, sg * 8:(sg + 1) * 8],
                                         rhs=q16[:, sg * 512:(sg + 1) * 512],
                                         start=(sg == 0), stop=(sg == 3))
                    if t % 2 == 0:
                        nc.vector.tensor_copy(
                            out=q4g[:, ti * 512:(ti + 1) * 512], in_=q4p[:])
                    else:
                        nc.scalar.activation(
                            q4g[:, ti * 512:(ti + 1) * 512], q4p[:], AF.Copy)
                qk = min(t0 // 12, 3)
                qt0 = TQ[qk][0]
                nc.sync.dma_start(
                    out=q4hbms[qk][:, (t0 - qt0) * 512:(t0 - qt0 + tn) * 512],
                    in_=q4g[:, :tn * 512])

            # ---- Q4 remap (per t-quarter tiles) + node phase, pipelined ----
            outT = apool.tile([64, D], dt.float32, tag="outT")
            for qk, (t0, tn) in enumerate(TQ):
                q4n = apool.tile([16, tn * 256], dt.bfloat16,
                                 tag=f"q4n{t0}")
                for r in range(2):
                    for i in range(4):
                        src = q4hbms[qk][:].rearrange(
                            "(h r) (t i bg) -> h r t i bg",
                            h=4, r=2, t=tn, i=4, bg=128)[:, r, :, i, :].squeeze()
                        dst = q4n[:].rearrange(
                            "(i h) (t r bg) -> i h t r bg",
                            i=4, h=4, t=tn, r=2, bg=128)[i, :, :, r, :].squeeze()
                        nc.gpsimd.dma_start(out=dst, in_=src)
                qw = tn * 256
                qchunks = [(c * 512, min(512, qw - c * 512))
                           for c in range((qw + 511) // 512)]
                for (o, w) in qchunks:
                    preP = psN.tile([64, 512], dt.float32, tag="preP", space="PSUM")
                    s64P = psN.tile([64, 512], dt.float32, tag="s64P", space="PSUM")
                    nc.tensor.matmul(out=preP[:, :w], lhsT=wrs[:, :64],
                                     rhs=q4n[:, o:o + w], start=True, stop=True)
                    nc.tensor.matmul(out=s64P[:, :w], lhsT=wrs[:, 64:],
                                     rhs=q4n[:, o:o + w], start=True, stop=True)
                    sv = wpool.tile([64, 512], dt.bfloat16, tag="sv")
                    act_recip(nc, sv[:, :w], s64P[:, :w])
                    pv = wpool.tile([64, 512], dt.bfloat16, tag="pv")
                    nc.scalar.activation(pv[:, :w], preP[:, :w], AF.Copy)
                    nc.vector.tensor_tensor(
                        out=outT[:, t0 * 256 + o:t0 * 256 + o + w],
                        in0=pv[:, :w], in1=sv[:, :w], op=mybir.AluOpType.mult)

            # ---- graph max-pool: per-graph reduce straight from outT ----
            pooled = wpool.tile([64, 64], dt.float32, tag="pooled")
            for gg in range(64):
                st, en = int(gstarts[gg]), int(gstarts[gg + 1])
                nc.vector.tensor_reduce(out=pooled[:, gg:gg + 1],
                                        in_=outT[:, st:en],
                                        axis=mybir.AxisListType.X,
                                        op=mybir.AluOpType.max)

            nc.scalar.activation(pooled[:], pooled[:], AF.Relu, biasv[:], 1.0)

            # ---- classifier ----
            clfp = psB.tile([64, 2], dt.float32, tag="q4p", space="PSUM")
            nc.tensor.matmul(out=clfp[:], lhsT=pooled[:], rhs=clfw[:],
                             start=True, stop=True)
            res = wpool.tile([64, 2], dt.float32, tag="res")
            nc.vector.tensor_tensor(out=res[:], in0=clfp[:], in1=clfb[:],
                                    op=mybir.AluOpType.add)
            nc.sync.dma_start(out=out_d[:], in_=res[:])

    nc.compile()
    return nc


# cell_flat = t*256 + b*32 + g*2 + r  <->  node n = t*256 + (b//2)*64 + r*32 + (b%2)*16 + g
_cf = np.arange(D)
_t, _m = _cf // 256, _cf % 256
_b, _g, _r = _m // 32, (_m % 32) // 2, _m % 2
NODE_OF_CELL = _t * 256 + _r * 128 + _b * 16 + _g


def _preprocess(x, src_all, dst_all):
    def _core(c):
        lo = c * DSH
        sel = (dst_all >= lo) & (dst_all < lo + DSH)
        srcs = src_all[sel]
        dsts = dst_all[sel] - lo
        deg = np.bincount(dsts, minlength=D)
        if deg.max() > L:
            print(f"kernel: core {c} max degree {deg.max()} > {L}; dropping excess")
        order = np.argsort(dsts, kind='stable')
        srcs, dsts = srcs[order], dsts[order]
        offs = np.zeros(D + 1, np.int64)
        offs[1:] = np.cumsum(deg)
        pos = np.arange(len(dsts)) - offs[dsts]
        keep = pos < L
        SRC = np.full((D, L), -1, np.int64)
        SRC[dsts[keep], pos[keep]] = srcs[keep]
        SRC = SRC[NODE_OF_CELL]                      # cell-indexed grid

        valid = SRC >= 0
        xs4 = np.zeros((D, L, 4), np.float32)
        xd4 = np.zeros((D, L, 4), np.float32)
        xs4[valid, :3] = x[SRC[valid]]
        xs4[valid, 3] = 1.0
        dnode = np.minimum(NODE_OF_CELL + lo, N - 1)
        dgrid = np.broadcast_to(dnode[:, None], (D, L))
        xd4[valid, :3] = x[dgrid[valid]]
        xd4[~valid, 3] = 1.0

        x6s = xs4.reshape(T, 8, 16, 2, L, 4)   # [t,b,g,r,l,i]
        x6d = xd4.reshape(T, 8, 16, 2, L, 4)
        XS2 = np.transpose(x6s, (0, 5, 2, 1, 3, 4)).reshape(T, 64, 1024)
        XD2 = np.transpose(x6d, (0, 5, 2, 1, 3, 4)).reshape(T, 64, 1024)
        xsdt = np.ascontiguousarray(
            np.concatenate([XS2, XD2], axis=1)).astype(BF)            # [T,128,1024]
        x4st = np.ascontiguousarray(
            np.transpose(x6s, (0, 3, 4, 5, 1, 2)).reshape(T, 128, 512)).astype(BF)
        return {"xsdt": xsdt, "x4st": x4st}

    return [_core(c) for c in range(NC)]


def kernel(feature_matrix, edge_index, batch, W, att_src, att_dst, bias, clf_W, clf_b):
    global _PROG
    from concourse.bass_utils import run_bass_kernel_spmd

    x = np.asarray(feature_matrix, dtype=np.float32)
    ei = np.asarray(edge_index).astype(np.int64)
    batch = np.asarray(batch).astype(np.int64)
    W = np.asarray(W, dtype=np.float32)
    att_src = np.asarray(att_src, dtype=np.float32)
    att_dst = np.asarray(att_dst, dtype=np.float32)
    bias = np.asarray(bias, dtype=np.float32)
    clf_W = np.asarray(clf_W, dtype=np.float32)
    clf_b = np.asarray(clf_b, dtype=np.float32)

    ar = np.arange(N, dtype=np.int64)
    src_all = np.concatenate([ei[0], ar])
    dst_all = np.concatenate([ei[1], ar])

    starts = np.searchsorted(batch, np.arange(G + 1, dtype=np.int64), side='left')
    gstarts = starts[:65].copy()
    for c in range(NC):
        sl = starts[64 * c:64 * c + 65] - c * DSH
        assert sl[0] == 0 and sl[64] == DSH, "graph/shard misalignment"
        assert np.array_equal(sl, gstarts), "graph pattern differs across cores"
    assert (gstarts[1:] - gstarts[:-1]).max() <= 256

    As3 = np.einsum('ihc,hc->ih', W.reshape(3, H, C), att_src)
    Ad3 = np.einsum('ihc,hc->ih', W.reshape(3, H, C), att_dst)
    As4 = np.concatenate([As3, np.zeros((1, H), np.float32)], 0)
    Ad4 = np.concatenate([Ad3, np.full((1, H), -100.0, np.float32)], 0)
    rhse4 = np.zeros((128, 64), np.float32)
    for s in range(2):
        A = As4 if s == 0 else Ad4
        for i in range(4):
            for g in range(16):
                rhse4[s * 64 + i * 16 + g, g * 4:g * 4 + 4] = A[i]
    selt = np.zeros((128, 32), np.float32)
    for sg in range(4):
        for n in range(128):
            selt[n, sg * 8 + 2 * sg + n // 64] = 1.0
    wrs = np.zeros((16, 128), np.float32)
    for h in range(4):
        for i in range(3):
            wrs[i * 4 + h, h * 16:(h + 1) * 16] = W.reshape(3, 4, 16)[i, h]
        wrs[3 * 4 + h, 64 + h * 16:64 + (h + 1) * 16] = 1.0
    consts = {
        "rhse4": rhse4.astype(BF), "selt": selt.astype(BF),
        "wrs": wrs.astype(BF),
        "biasv": bias.reshape(64, 1).astype(np.float32),
        "clfw": clf_W.astype(np.float32),
        "clfb": np.broadcast_to(clf_b[None, :], (64, 2)).astype(np.float32).copy(),
    }

    if _PROG is None:
        _PROG = _build_program(gstarts)
    nc = _PROG

    per_core = _preprocess(x, src_all, dst_all)
    in_maps = [{**pc, **consts} for pc in per_core]
    res = run_bass_kernel_spmd(nc, in_maps, list(range(NC)))
    out = np.concatenate([res.results[c]["out"] for c in range(NC)], axis=0)
    return out.astype(np.float32)
